# revision 36
# baseline (speedup 1.0000x reference)
"""CLIP-style attention with MULT-expanded K/V (nn_CLIPAttentionMKV) on 8
Trainium2 NeuronCores.

Sharding: core = (batch b, head-group g); 4 batches x 2 groups of 8 heads.
Each core computes its batch's Q/K/V projections for its 8 heads, the
per-head attention, and a partial output projection (contracting over its
512 of the 1024 hidden features).  The partials leave the device as fp16
[E, T] tiles; the host sums the two partials per batch in fp32.

Speed recipe (cost-model-driven; matmul cost = moving-free-size x
cycles-per-row, independent of the contraction/stationary dims):
  * Q/K/V projections run as fp8e4m3 DoubleRow matmuls (0.5 cycles/row,
    2 k-tiles per instruction, so 8 f32r k-tile passes become 12
    half-cost ones = 25% off) with an error-compensated hi+lo split:
    W ~= Wh + Wl, x ~= xh + xl, proj ~= Wh.xh + Wh.xl + Wl.xh accumulated
    in one PSUM group.  Weights are pre-scaled by 64 on host (undone in
    the PSUM->SBUF store) so W's 0.02-scale entries stay out of fp8's
    subnormal range.  End-to-end error ~2.3e-3 vs the 2e-2 gate.
  * Scores stay fp16xfp16: one 512-row matmul per (i, head) already does
    the full 64-deep contraction, so fp8 DoubleRow can't beat it without
    dropping to plain fp8 (measured ~1.9e-2 - too close to the gate).
    exp runs on ACT straight out of PSUM.
  * AV is "flipped": out[t,f] += e[s,t]^T @ v[s,f] with the 65-wide
    (64 v-cols + ones column for the softmax normalizer Z) moving
    operand, so each matmul costs 65 rows instead of 512.  The per-t Z
    lands on the partition axis, making the normalize a native
    per-partition broadcast multiply on DVE (no gpsimd broadcast).
  * The normalized [t,f] tile is transposed back to [f,t] with 4 PE
    transpose matmuls per head-pair (fp16 identity) for the output
    projection, which keeps its [f,t] fp16 moving layout.
  * Scheduling: the kernel is globally PE-bound (~150us of PE work vs
    ~134us of ACT exp), so every projection / output-projection PSUM
    group is a named ~0.6-1.3us "unit" pulled either just-in-time
    (right before the attention step that first reads it, including
    next-pair prefetch at i=11/12) or by a per-step PE-slack budget.
    AV matmuls trail their exp by LAG i-steps so the PE never waits on
    ACT; pair (0,0) streams its own v/k units one step ahead of use so
    the first exp fires ~10us in; outproj for tau=0 back-fills the last
    two pairs, and the tau=1 tail runs its bias adds on the idle ACT
    (alternating with DVE) with output DMAs batched 4 feature-chunks
    per descriptor set.
"""

import numpy as np
import ml_dtypes

import concourse.bacc as bacc
import concourse.bass as bass
import concourse.mybir as mybir
import concourse.tile as tile
from concourse import bass_utils
from concourse.bass import ts

B, T, E = 4, 1024, 1024
H, MULT = 16, 2
HD = E // H            # 64
S = T * MULT           # 2048
SCALE = HD ** -0.5
P = 128
G = 2                  # head groups == cores per batch
HG = H // G            # 8 heads per group
FG = HG * HD           # 512 q features per group
F2 = MULT * FG         # 1024 k/v features per group
N_CORES = B * G
NT = 512               # matmul moving free dim
KO = E // P            # 8 contraction k-tiles for projections
KOP = KO // 2          # 4 DoubleRow k-tile pairs
NJQ = FG // P          # 4 q-feature chunks
NJK = F2 // P          # 8 k-feature chunks
VCH = 256              # v-proj psum chunk (4 head-blocks of 64)
NPH = F2 // VCH        # 4 v-proj chunks
WSC = 64.0             # host premultiplier on fp8 weights
WINV = float(1.0 / WSC)

F32 = mybir.dt.float32
F32R = mybir.dt.float32r
F16 = mybir.dt.float16
F8 = mybir.dt.float8e4
ADD = mybir.AluOpType.add
MUL = mybir.AluOpType.mult
EXP = mybir.ActivationFunctionType.Exp
IDN_F = mybir.ActivationFunctionType.Identity
DR = mybir.MatmulPerfMode.DoubleRow

_compiled = {}


def _build():
    nc = bacc.Bacc("TRN2", target_bir_lowering=False, debug=False,
                   num_devices=N_CORES)
    xh = nc.dram_tensor("xh", [P, KO, T], F8, kind="ExternalInput").ap()
    xl = nc.dram_tensor("xl", [P, KO, T], F8, kind="ExternalInput").ap()
    wqd = nc.dram_tensor("wqd", [NJQ, P, KO, 2, P], F8,
                         kind="ExternalInput").ap()
    wkd = nc.dram_tensor("wkd", [NJK, P, KO, 2, P], F8,
                         kind="ExternalInput").ap()
    wvd = nc.dram_tensor("wvd", [NPH, P, KO, 2, VCH], F8,
                         kind="ExternalInput").ap()
    wod = nc.dram_tensor("wod", [E // P, P, FG // P, P], F16,
                         kind="ExternalInput").ap()
    idn_d = nc.dram_tensor("idn", [P, P], F16, kind="ExternalInput").ap()
    bq = nc.dram_tensor("bq", [FG], F32, kind="ExternalInput").ap()
    bk = nc.dram_tensor("bk", [F2], F32, kind="ExternalInput").ap()
    bo = nc.dram_tensor("bo", [E], F32, kind="ExternalInput").ap()
    out = nc.dram_tensor("out", [E, T], F16, kind="ExternalOutput").ap()

    with tile.TileContext(nc) as tc:
        with (
            tc.tile_pool(name="resident", bufs=1) as res,
            # one PSUM pool; tag bank budget: qk 2x2 + av0 1 + av1 1 +
            # mm 2 = 8.  Every tag's tiles are sized 2KB/partition (qk
            # 4KB) so the slot size stays tag-consistent.
            tc.tile_pool(name="psum", bufs=1, space="PSUM") as psum,
            tc.tile_pool(name="wqk", bufs=6) as wp,
            tc.tile_pool(name="epool", bufs=4) as ep,
            tc.tile_pool(name="rpool", bufs=2) as rp,
            tc.tile_pool(name="apool", bufs=2) as app,
            tc.tile_pool(name="osb", bufs=3) as ob,
        ):
            # ---- resident tiles ----
            q_sb = res.tile([P, NJQ, T], F16)       # q^T  [f, t]
            kfeat = res.tile([P, NJK, T], F16)      # k^T  [f, t]
            vaug = res.tile([P, T // P, F2 + MULT * HG], F16)  # v [t, faug]
            attn_out = res.tile([P, NJQ, T], F16)   # attnout^T [f, t]
            # tau=1 outproj partial sums (ko 0..2), built during the last
            # pair so only the ko=3 matmul + combine trail the final exp
            part = res.tile([P, E // P, NT], F16)
            idn = res.tile([P, P], F16)
            bq_sb = res.tile([P, NJQ], F32)
            bk_sb = res.tile([P, NJK], F32)
            bo_sb = res.tile([P, E // P], F32)

            out3 = out.rearrange("(jo p) t -> p jo t", p=P)

            with tc.tile_pool(name="p1big", bufs=1) as p1:
                xh_sb = p1.tile([P, KO, T], F8)
                xl_sb = p1.tile([P, KO, T], F8)

                def qk_w(wd, j, nm, eng=None):
                    wt = wp.tile([P, KO, 2, P], F8, tag="wqk", bufs=6,
                                 name=f"wt_{nm}{j}")
                    (eng or nc.sync).dma_start(wt[:], wd[j])
                    return wt

                def dr_passes():
                    # hi*hi, lo*hi, hi*lo; lo*lo is dropped (O(eps^2)).
                    return ((xh_sb, 0), (xl_sb, 0), (xh_sb, 1))

                def qk_proj(j, tau, b_sb, o_sb, wt, ptag="mm", pbufs=2):
                    pt = psum.tile([P, NT], F32, tag=ptag, bufs=pbufs,
                                   name=f"pj_{o_sb.name}_{j}_{tau}")
                    first = True
                    for xs, w in dr_passes():
                        for kop in range(KOP):
                            nc.tensor.matmul(
                                pt[:], wt[:, 2 * kop:2 * kop + 2, w],
                                xs[:, 2 * kop:2 * kop + 2, ts(tau, NT)],
                                start=first,
                                stop=(w == 1 and kop == KOP - 1),
                                perf_mode=DR)
                            first = False
                    nc.vector.scalar_tensor_tensor(
                        o_sb[:, j, ts(tau, NT)], pt[:], WINV,
                        b_sb[:, j:j + 1].to_broadcast((P, NT)), MUL, ADD)

                def v_w(phi, eng=None):
                    wvt = wp.tile([P, KO, 2, VCH], F8, tag="wv", bufs=4,
                                  name=f"wvt{phi}")
                    (eng or nc.sync).dma_start(wvt[:], wvd[phi])
                    return wvt

                def v_proj_i(phi, i, wvt):
                    pt = psum.tile([P, NT], F32, tag="mm", bufs=2,
                                   name=f"pv_{phi}_{i}")
                    first = True
                    for xs, w in dr_passes():
                        for kop in range(KOP):
                            nc.tensor.matmul(
                                pt[:, 0:VCH],
                                xs[:, 2 * kop:2 * kop + 2, ts(i, P)],
                                wvt[:, 2 * kop:2 * kop + 2, w],
                                start=first,
                                stop=(w == 1 and kop == KOP - 1),
                                perf_mode=DR)
                            first = False
                    # scatter the 4 head-blocks into the 65-stride augmented
                    # layout (ones columns skipped), undoing the x64 weight
                    # prescale.
                    dst = vaug[:, i, ts(phi, VCH + 4)].rearrange(
                        "p (b c) -> p b c", c=HD + 1)
                    nc.vector.tensor_scalar(
                        dst[:, :, 0:HD],
                        pt[:, 0:VCH].rearrange("p (b c) -> p b c", c=HD),
                        WINV, None, MUL)

                def wo_w(j, tau):
                    wot = wp.tile([P, FG // P, P], F16, tag="wo", bufs=8,
                                  name=f"wot{j}_{tau}")
                    nc.sync.dma_start(wot[:], wod[j])
                    return wot

                ostage = {}

                def outproj_j(j, tau, wot, ptag="mm", pbufs=2,
                              bias_on_act=False):
                    pt = psum.tile([P, NT], F32, tag=ptag, bufs=pbufs,
                                   name=f"po_{j}_{tau}")
                    for ko in range(FG // P):
                        nc.tensor.matmul(
                            pt[:], wot[:, ko], attn_out[:, ko, ts(tau, NT)],
                            start=(ko == 0), stop=(ko == FG // P - 1))
                    # out tiles are staged 4 js per SBUF tile so one DMA
                    # covers 4 feature chunks (DMA issues cost ~1.5us of
                    # sequencer time each - fewer is faster at the tail).
                    grp = j // 4
                    if (tau, grp) not in ostage:
                        ostage[(tau, grp)] = ob.tile(
                            [P, 4, NT], F16, tag="ot", bufs=2,
                            name=f"ot_{tau}_{grp}")
                    ot = ostage[(tau, grp)]
                    if bias_on_act and (j % 2):
                        # tail path: ACT is idle once the exps are done;
                        # alternating ACT/DVE halves the serial bias chain
                        nc.scalar.activation(ot[:, j % 4], pt[:], IDN_F,
                                             bias=bo_sb[:, j:j + 1])
                    else:
                        nc.vector.tensor_tensor(
                            ot[:, j % 4], pt[:],
                            bo_sb[:, j:j + 1].to_broadcast((P, NT)), ADD)
                    if j % 4 == 3:
                        nc.sync.dma_start(
                            out3[:, 4 * grp:4 * grp + 4, ts(tau, NT)], ot[:])

                def outproj_part_j(j, wot):
                    pp = psum.tile([P, NT], F32, tag="mm", bufs=2,
                                   name=f"pp_{j}")
                    for ko in range(3):
                        nc.tensor.matmul(
                            pp[:], wot[:, ko], attn_out[:, ko, ts(1, NT)],
                            start=(ko == 0), stop=(ko == 2))
                    nc.vector.tensor_copy(part[:, j], pp[:])

                def outproj_ko3_j(j, wot, ptag, pbufs):
                    pt = psum.tile([P, NT], F32, tag=ptag, bufs=pbufs,
                                   name=f"p3_{j}")
                    nc.tensor.matmul(pt[:], wot[:, 3],
                                     attn_out[:, 3, ts(1, NT)],
                                     start=True, stop=False)
                    # accumulate the ko0..2 partial into PSUM on the PE
                    # itself (identity stationary), so the tail needs no
                    # serial DVE combine chain - only the ACT bias-add
                    nc.tensor.matmul(pt[:], idn[:], part[:, j],
                                     start=False, stop=True)
                    grp = j // 4
                    if (1, grp) not in ostage:
                        ostage[(1, grp)] = ob.tile(
                            [P, 4, NT], F16, tag="ot", bufs=2,
                            name=f"ot_1_{grp}")
                    ot = ostage[(1, grp)]
                    # ACT and DVE alternate so the serial bias chain halves
                    if j % 2:
                        nc.scalar.activation(ot[:, j % 4], pt[:], IDN_F,
                                             bias=bo_sb[:, j:j + 1])
                    else:
                        nc.vector.tensor_tensor(
                            ot[:, j % 4], pt[:],
                            bo_sb[:, j:j + 1].to_broadcast((P, NT)), ADD)
                    if j % 4 == 3:
                        nc.sync.dma_start(
                            out3[:, 4 * grp:4 * grp + 4, ts(1, NT)], ot[:])

                # ---- attention: pre_step(i) runs at the top of every
                # i-iteration so projection units can be interleaved.  AV
                # matmuls are emitted LAG i-steps behind their exp so the PE
                # never stalls waiting for ACT (the kernel is globally
                # PE-bound; any PE wait is lost wall-clock). ----
                LAG = 4

                def attn_pair(tau, hp, pre_step):
                    avs = [None, None]
                    av3 = [None, None]
                    pend = []

                    def emit_av(rec):
                        i, mu, tpt, et = rec
                        for hh in range(2):
                            vcol = (mu * HG + hp * 2 + hh) * (HD + 1)
                            for tsub in range(4):
                                nc.tensor.matmul(
                                    av3[hh][:, tsub],
                                    et[:, hh * NT + tsub * P:
                                       hh * NT + (tsub + 1) * P],
                                    vaug[:, tpt, vcol:vcol + HD + 1],
                                    start=(i == 0 and tsub == 0),
                                    stop=(i == S // P - 1 and tsub == 3))

                    for i in range(S // P):
                        pre_step(i)
                        if i == 0:
                            for hh in range(2):
                                avs[hh] = psum.tile(
                                    [P, NT], F32, tag=f"av{hh}", bufs=1,
                                    name=f"av_{tau}_{hp}_{hh}")
                                av3[hh] = avs[hh][:, 0:4 * (HD + 1)].rearrange(
                                    "p (b c) -> p b c", c=HD + 1)
                        mu, tpt = divmod(i, T // P)
                        qk = psum.tile([P, 2 * NT], F32, tag="qk", bufs=2,
                                       name=f"qk_{tau}_{hp}_{i}")
                        for hh in range(2):
                            base = hh * HD
                            fo = mu * NJQ + hp
                            nc.tensor.matmul(
                                qk[:, ts(hh, NT)],
                                kfeat[base:base + HD, fo, ts(tpt, P)],
                                q_sb[base:base + HD, hp, ts(tau, NT)],
                                start=True, stop=True)
                        et = ep.tile([P, 2 * NT], F16, tag="e", bufs=8,
                                     name=f"e_{tau}_{hp}_{i}")
                        nc.scalar.activation(et[:], qk[:], EXP)
                        pend.append((i, mu, tpt, et))
                        if len(pend) > LAG:
                            emit_av(pend.pop(0))
                    while pend:
                        emit_av(pend.pop(0))
                    pre_step(S // P)
                    atf = app.tile([P, 4, 2 * HD], F16, tag="atf", bufs=2,
                                   name=f"atf_{tau}_{hp}")
                    for hh in range(2):
                        rec = rp.tile([P, 4, 1], F32, tag="rec", bufs=2,
                                      name=f"rec_{tau}_{hp}_{hh}")
                        nc.vector.reciprocal(rec[:], av3[hh][:, :, HD:HD + 1])
                        nc.vector.tensor_tensor(
                            atf[:, :, hh * HD:(hh + 1) * HD],
                            av3[hh][:, :, 0:HD],
                            rec[:].to_broadcast((P, 4, HD)), MUL)
                    # transpose [t, f] -> [f, t] via 4 PE transposes into one
                    # (recycled av0) PSUM bank, then one DVE copy out.
                    tp = psum.tile([P, 2 * NT], F16, tag="av0", bufs=1,
                                   name=f"tp_{tau}_{hp}")
                    for tsub in range(4):
                        nc.tensor.matmul(
                            tp[:, ts(tsub, P)], atf[:, tsub], idn[:],
                            is_transpose=True,
                            start=(tsub == 0), stop=(tsub == 3))
                    nc.vector.tensor_copy(attn_out[:, hp, ts(tau, NT)],
                                          tp[:, 0:NT])

                # ---- startup DMAs.  The DMA engines are a shared pipe,
                # so order by first use: the t<512 halves of x-hi/x-lo plus
                # wq0/wk0/wv0 unblock unit q00 / k00 / v0_0 and the first
                # exp ~7us in; everything else streams behind. ----
                nc.sync.dma_start(idn[:], idn_d[:])
                nc.scalar.dma_start(xh_sb[:, :, 0:NT], xh[:, :, 0:NT])
                wt_q0 = qk_w(wqd, 0, "q")
                nc.scalar.dma_start(xl_sb[:, :, 0:NT], xl[:, :, 0:NT])
                wt_k0 = qk_w(wkd, 0, "k")
                nc.sync.dma_start(bq_sb[:], bq.rearrange("(o p) -> p o", p=P))
                nc.sync.dma_start(bk_sb[:], bk.rearrange("(o p) -> p o", p=P))
                wvt0 = v_w(0)
                nc.scalar.dma_start(xh_sb[:, :, NT:T], xh[:, :, NT:T])
                nc.scalar.dma_start(xl_sb[:, :, NT:T], xl[:, :, NT:T])
                wt_k4 = qk_w(wkd, 4, "k")
                wvt2 = v_w(2)
                nc.sync.dma_start(bo_sb[:], bo.rearrange("(o p) -> p o", p=P))
                onesf = p1.tile([P, T // P, MULT * HG], F16, name="onesf")
                nc.gpsimd.memset(onesf[:], 1.0)
                va5 = vaug.rearrange("p i (b c) -> p i b c", c=HD + 1)
                nc.vector.tensor_copy(va5[:, :, :, HD:HD + 1], onesf[:])
                # PE p-state warm-up: ~2us of back-to-back identity
                # transposes into a scratch bank while the x DMAs land, so
                # the first projection runs at the full 2.4GHz clock.
                warm = psum.tile([P, 2 * NT], F16, tag="av1", bufs=1,
                                 name="warm")
                for w in range(40):
                    nc.tensor.matmul(
                        warm[:, ts(w % 4, P)], idn[:], idn[:],
                        is_transpose=True,
                        start=(w < 4), stop=(w >= 36))

                # ---- unit registry: every projection / output-projection
                # PSUM-tile group is a named unit.  Units are pulled either
                # just-in-time (forced, right before the attention step that
                # first reads their output) or by a per-step PE-slack budget
                # walking the deferred order.  done-flags make the pulls
                # order-independent. ----
                wt_q1 = qk_w(wqd, 1, "q")
                wt_k1 = qk_w(wkd, 1, "k")
                wt_k5 = qk_w(wkd, 5, "k")
                wvt1 = v_w(1)
                wvt3 = v_w(3)
                wt_q2 = qk_w(wqd, 2, "q")
                wt_k2 = qk_w(wkd, 2, "k")
                wt_k6 = qk_w(wkd, 6, "k")
                wt_q3 = qk_w(wqd, 3, "q")
                wt_k3 = qk_w(wkd, 3, "k")
                wt_k7 = qk_w(wkd, 7, "k")
                wqt = {0: wt_q0, 1: wt_q1, 2: wt_q2, 3: wt_q3}
                wkt = {0: wt_k0, 1: wt_k1, 2: wt_k2, 3: wt_k3,
                       4: wt_k4, 5: wt_k5, 6: wt_k6, 7: wt_k7}
                wvt = {0: wvt0, 1: wvt1, 2: wvt2, 3: wvt3}

                units = {}
                order = []
                done = set()

                def U(name, cost, fn, *a):
                    units[name] = (cost, lambda: fn(*a))
                    order.append(name)

                QC, VC, OC = 1280.0, 640.0, 850.0
                for hp in range(NJQ):
                    for tau in range(2):
                        U(f"q{hp}{tau}", QC, qk_proj, hp, tau,
                          bq_sb, q_sb, wqt[hp])
                for j in range(NJK):
                    for tau in range(2):
                        U(f"k{j}{tau}", QC, qk_proj, j, tau,
                          bk_sb, kfeat, wkt[j])
                for phi in range(NPH):
                    for t in range(T // P):
                        U(f"v{phi}_{t}", VC, v_proj_i, phi, t, wvt[phi])

                debt = [0.0]

                def pull(name):
                    if name in done:
                        return
                    done.add(name)
                    c, th = units[name]
                    th()
                    debt[0] -= c

                deferred = ["q00", "k00"]
                for t in range(4):
                    deferred.append(f"v0_{t}")
                deferred.append("k01")
                for t in range(4, T // P):
                    deferred.append(f"v0_{t}")
                deferred.append("k40")
                for t in range(4):
                    deferred.append(f"v2_{t}")
                deferred.append("k41")
                for t in range(4, T // P):
                    deferred.append(f"v2_{t}")
                deferred += ["q01", "q10", "k10", "k11", "k50", "k51", "q11"]
                for t in range(T // P):
                    deferred.append(f"v1_{t}")
                deferred += ["q20", "k20", "k21", "k60", "k61"]
                for t in range(4):
                    deferred.append(f"v3_{t}")
                deferred += ["q30", "k30", "k31", "k70", "k71"]
                for t in range(4, T // P):
                    deferred.append(f"v3_{t}")
                deferred += ["q21", "q31"]
                didx = [0]

                def fill_budget(ns):
                    debt[0] += ns
                    while debt[0] > 0 and didx[0] < len(deferred):
                        nm = deferred[didx[0]]
                        didx[0] += 1
                        pull(nm)

                pair_order = [(0, 0), (1, 0), (0, 1), (1, 1),
                              (0, 2), (0, 3), (1, 2), (1, 3)]
                wo_tiles = {}

                def mk_pre_step(pi, tau, hp, nxt):
                    vphi = (1, 3) if hp >= 2 else (0, 2)

                    def ps(i):
                        if i < S // P:
                            if i == 0:
                                pull(f"q{hp}{tau}")
                                pull(f"k{hp}0")
                            if i == 4:
                                pull(f"k{hp}1")
                            if i == 8:
                                pull(f"k{hp + 4}0")
                            if i == 12:
                                pull(f"k{hp + 4}1")
                            # next pair's first units land before its QK_0
                            # so the pair boundary has no forced block
                            if nxt is not None:
                                if i == 13:
                                    pull(f"q{nxt[1]}{nxt[0]}")
                                if i == 14:
                                    pull(f"k{nxt[1]}0")
                        # v chunk feeding the one-step-later (lagged) AV;
                        # pulled after this step's QK so the exp chain isn't
                        # delayed at startup.  The final chunk must land at
                        # i=15: the AV flush (which reads it) runs before
                        # pre_step(16).
                        if 1 <= i < S // P:
                            mu, tpt = divmod(i - 1, T // P)
                            pull(f"v{vphi[mu]}_{tpt}")
                            if i == S // P - 1:
                                pull(f"v{vphi[1]}_{T // P - 1}")
                        if i == S // P:
                            # a unit between the AV flush and the transposes
                            # hides the normalize (DVE) latency from the PE
                            if nxt is not None:
                                pull(f"k{nxt[1]}1")
                        if pi == 0 and i < 2:
                            # don't let budget filler (v0_0, whose weights
                            # are still in flight) delay the very first QK
                            return
                        fill_budget((400.0, 900.0)[pi - 6] if pi >= 6
                                    else 380.0)
                    return ps

                for pi, (tau, hp) in enumerate(pair_order):
                    debt[0] = min(debt[0], 0.0)
                    if pi == 5:
                        for j in range(E // P):
                            wo_tiles[j] = wo_w(j, 0)
                    if pi == 6:
                        # attn_out tau=0 complete: its outproj becomes
                        # budget-filler inside the last two pairs
                        for j in range(E // P):
                            nm = f"o{j}_0"
                            units[nm] = (OC, (lambda jj: lambda:
                                          outproj_j(jj, 0, wo_tiles[jj]))(j))
                            deferred.append(nm)
                    if pi == 7:
                        # attn_out tau=1 ko 0..2 complete: partial outproj
                        # sums fill the last pair's dry steps
                        for j in range(E // P):
                            nm = f"p{j}_1"
                            units[nm] = (OC, (lambda jj: lambda:
                                          outproj_part_j(jj, wo_tiles[jj]))(j))
                            deferred.append(nm)
                    nxt = pair_order[pi + 1] if pi + 1 < 8 else None
                    attn_pair(tau, hp, mk_pre_step(pi, tau, hp, nxt))
                while didx[0] < len(deferred):
                    nm = deferred[didx[0]]
                    didx[0] += 1
                    pull(nm)
                # tail: only the ko=3 slice of outproj tau=1 remains,
                # cycling the fully-retired qk/mm psum slots
                tags = [("qk", 2), ("qk", 2), ("mm", 2), ("mm", 2)]
                for j in range(E // P):
                    tg, tb = tags[j % len(tags)]
                    outproj_ko3_j(j, wo_tiles[j], tg, tb)

    nc.compile()
    return nc


def _get_compiled(aug=False):
    if "nc" not in _compiled:
        _compiled["nc"] = _build()
    return _compiled["nc"]


def _numpy_reference(hidden_states, attention_mask, Wq, bq, Wk, bk, Wv, bv,
                     Wo, bo):
    """Exact fp32 fallback (used only for nonzero mask / bv)."""
    x = hidden_states
    q = (np.einsum("bte,fe->btf", x, Wq) + bq) * SCALE
    q = q.reshape(B, T, H, HD).transpose(0, 2, 1, 3)
    k = (np.einsum("bte,fe->btf", x, Wk) + bk).reshape(B, S, H, HD)
    k = k.transpose(0, 2, 1, 3)
    v = (np.einsum("bte,fe->btf", x, Wv) + bv).reshape(B, S, H, HD)
    v = v.transpose(0, 2, 1, 3)
    attn = np.einsum("bhtd,bhsd->bhts", q, k)
    attn = attn.reshape(B, H, T, MULT, T) + attention_mask[:, :, :, None, :]
    attn = attn.reshape(B, H, T, S)
    attn = attn - attn.max(-1, keepdims=True)
    attn = np.exp(attn)
    attn /= attn.sum(-1, keepdims=True)
    out = np.einsum("bhts,bhsd->bhtd", attn, v)
    out = out.transpose(0, 2, 1, 3).reshape(B, T, E)
    return (np.einsum("bte,fe->btf", out, Wo) + bo).astype(np.float32)


F8NP = ml_dtypes.float8_e4m3


def _hi_lo(a):
    hi = a.astype(F8NP)
    lo = (a - hi.astype(np.float32)).astype(F8NP)
    return hi, lo


def _pack_w(wT, nj):
    """[E, F] f32 (pre-scaled) -> [nj, P, KO, 2, F//nj] fp8 hi/lo tiles."""
    Ei, F = wT.shape
    hi, lo = _hi_lo(wT)
    w = np.stack([hi, lo], axis=1)            # [E, 2, F]
    w = w.reshape(KO, P, 2, nj, F // nj)      # e=(ko p), f=(j fj)
    return np.ascontiguousarray(w.transpose(3, 1, 0, 2, 4))


def kernel(hidden_states, attention_mask, Wq, bq, Wk, bk, Wv, bv, Wo, bo):
    hidden_states = np.asarray(hidden_states, dtype=np.float32)
    attention_mask = np.asarray(attention_mask, dtype=np.float32)
    Wq = np.asarray(Wq, dtype=np.float32)
    bq = np.asarray(bq, dtype=np.float32)
    Wk = np.asarray(Wk, dtype=np.float32)
    bk = np.asarray(bk, dtype=np.float32)
    Wv = np.asarray(Wv, dtype=np.float32)
    bv = np.asarray(bv, dtype=np.float32)
    Wo = np.asarray(Wo, dtype=np.float32)
    bo = np.asarray(bo, dtype=np.float32)

    if attention_mask.any() or bv.any():
        # The TRN2 kernel folds the (always-zero) mask and v-bias away;
        # handle the general case exactly on host.
        return _numpy_reference(hidden_states, attention_mask, Wq, bq, Wk,
                                bk, Wv, bv, Wo, bo)

    nc = _get_compiled()

    idn = np.eye(P, dtype=np.float16)
    in_maps = []
    for core in range(N_CORES):
        b, g = divmod(core, G)
        rows = slice(g * FG, (g + 1) * FG)
        wk_g = np.concatenate(
            [Wk[m * E + g * FG: m * E + (g + 1) * FG] for m in range(MULT)], 0)
        bk_g = np.concatenate(
            [bk[m * E + g * FG: m * E + (g + 1) * FG] for m in range(MULT)], 0)
        wv_g = np.concatenate(
            [Wv[m * E + g * FG: m * E + (g + 1) * FG] for m in range(MULT)], 0)

        xT = np.ascontiguousarray(hidden_states[b].T)        # [E, T]
        xhi, xlo = _hi_lo(xT)
        xh = np.ascontiguousarray(
            xhi.reshape(KO, P, T).transpose(1, 0, 2))
        xl = np.ascontiguousarray(
            xlo.reshape(KO, P, T).transpose(1, 0, 2))

        wo_g = np.ascontiguousarray(Wo[:, rows].T)           # [FG, E]
        wod = np.ascontiguousarray(
            wo_g.reshape(NJQ, P, E // P, P).transpose(2, 1, 0, 3)
        ).astype(np.float16)

        in_maps.append({
            "xh": xh,
            "xl": xl,
            "wqd": _pack_w((Wq[rows] * SCALE).T * WSC, NJQ),
            "wkd": _pack_w(wk_g.T * WSC, NJK),
            "wvd": _pack_w(wv_g.T * WSC, NPH),
            "wod": wod,
            "idn": idn,
            "bq": np.ascontiguousarray(bq[rows] * SCALE),
            "bk": np.ascontiguousarray(bk_g),
            "bo": bo if g == 0 else np.zeros_like(bo),
        })

    res = bass_utils.run_bass_kernel_spmd(
        nc, in_maps, core_ids=list(range(N_CORES)))

    final = np.empty((B, T, E), dtype=np.float32)
    for b in range(B):
        acc = (res.results[G * b]["out"].astype(np.float32)
               + res.results[G * b + 1]["out"].astype(np.float32))
        final[b] = acc.T
    return final
